# revision 1
# baseline (speedup 1.0000x reference)
"""GCN layer (message passing + linear + ReLU) on 8 Trainium2 NeuronCores.

out = relu(((scatter_add(h[src] -> dst) + x) * dis) @ W.T),
h = x * dis,  dis = rsqrt(deg + 1),  deg = in-degree via dst counts.

Strategy (SPMD, one program on 8 cores):
  - Nodes sharded contiguously: core c owns rows [c*6250, (c+1)*6250).
  - Host partitions edges by dst owner and sorts by dst (index-only work);
    degree reaches the device as CSR rowptr slices, so deg = rowptr diff
    and dis = 1/sqrt(deg+1) are computed on device in f32.
  - Each core computes the full h table (f32, 512B rows) into its DRAM,
    then bulk-gathers h[src] for its own edges with dma_gather (SWDGE).
    int16 gather indices can't span 50k rows, so edges are processed in
    two passes against table halves h[:32768] / h[32768:].
  - Scatter-add is done on-chip: edges sorted by dst fall into windows of
    128 owned nodes; per 128-edge chunk a one-hot S[e, slot] matrix is
    built on DVE (slot = dst - window_base, 255 pads mask out) and the
    tensor engine accumulates S.T @ H_chunk into the window's PSUM tile.
  - Finalize per window: (+x, *dis), PE transpose, 128x128 linear with
    W.T, ReLU, store.
Chunk counts per (pass, window) are maxed over cores so the single SPMD
program fits every core; shorter cores pad with slot=255 chunks.
"""
import numpy as np

from concourse import bacc, bass, mybir, tile
from concourse.bass_utils import run_bass_kernel_spmd

F32 = mybir.dt.float32
I32 = mybir.dt.int32
I16 = mybir.dt.int16
AF = mybir.ActivationFunctionType
OP = mybir.AluOpType

N = 50000
E = 600000
D = 128
C = 8                      # cores
NPC = N // C               # 6250 nodes per core
WPC = (NPC + 127) // 128   # 49 windows per core
NPAD = WPC * 128           # 6272 padded shard rows
PT_G = (N + 127) // 128    # 391 global node tiles
NPAD_G = PT_G * 128        # 50048
SPLIT = 32768              # src table split for int16 gather indices
PASS_BOUNDS = [(0, SPLIT), (SPLIT, N)]
GB = 8                     # chunks per dma_gather batch (1024 idxs; >1024
                           # descriptors per SWDGE call crashes the device)
TB = 8                     # node tiles per h-pass DMA


# ---------------------------------------------------------------- host prep
def host_prep(edge_index):
    src = np.asarray(edge_index[0], dtype=np.int64)
    dst = np.asarray(edge_index[1], dtype=np.int64)
    order = np.argsort(dst, kind="stable")
    ss_all = src[order]
    dd_all = dst[order]
    counts = np.bincount(dst, minlength=N)
    rowptr = np.zeros(N + 1, np.int64)
    rowptr[1:] = np.cumsum(counts)

    rp = np.full(NPAD_G + 1, rowptr[N], np.int64)
    rp[: N + 1] = rowptr
    rp0g = rp[:NPAD_G].reshape(PT_G, 128).T.astype(np.int32).copy()
    rp1g = rp[1 : NPAD_G + 1].reshape(PT_G, 128).T.astype(np.int32).copy()

    per_core = []
    need = np.zeros((C, 2, WPC), np.int64)
    for c in range(C):
        e0, e1 = rowptr[c * NPC], rowptr[(c + 1) * NPC]
        ss, dd = ss_all[e0:e1], dd_all[e0:e1]
        per_core.append((ss, dd))
        for p, (lo, hi) in enumerate(PASS_BOUNDS):
            m = (ss >= lo) & (ss < hi)
            w = (dd[m] - c * NPC) // 128
            need[c, p] = np.bincount(w, minlength=WPC)
    K = np.ceil(need.max(axis=0) / 128).astype(np.int64)  # [2, WPC]
    CH = K.sum(axis=1)
    cstart = np.zeros((2, WPC), np.int64)
    for p in range(2):
        cstart[p, 1:] = np.cumsum(K[p][:-1])

    cores = []
    for c in range(C):
        ss, dd = per_core[c]
        d = {}
        for p, (lo, hi) in enumerate(PASS_BOUNDS):
            m = (ss >= lo) & (ss < hi)
            sp = (ss[m] - lo).astype(np.int64)
            dloc = dd[m] - c * NPC
            w = dloc // 128
            g = np.zeros(CH[p] * 128, np.int64)
            s = np.full(CH[p] * 128, 255, np.int64)
            cnt = np.bincount(w, minlength=WPC)
            ofs = np.zeros(WPC, np.int64)
            ofs[1:] = np.cumsum(cnt[:-1])
            pos = cstart[p, w] * 128 + (np.arange(len(sp)) - ofs[w])
            g[pos] = sp
            s[pos] = dloc - w * 128
            tag = "lo" if p == 0 else "hi"
            # gather idx layout [128, CH*8]: stream pos j at [j%16, j//16],
            # replicated across the 8 groups of 16 partitions.
            d[f"gidx_{tag}"] = np.tile(
                g.reshape(-1, 16).T.astype(np.int16), (8, 1)
            ).copy()
            # slot layout [128, CH]: stream pos j at [j%128, j//128]
            d[f"slots_{tag}"] = s.reshape(-1, 128).T.astype(np.int16).copy()
        n0 = c * NPC
        rpv = np.full(NPAD + 1, rowptr[min((c + 1) * NPC, N)], np.int64)
        rpv[: NPC + 1] = rowptr[n0 : n0 + NPC + 1]
        d["rp0s"] = rpv[:NPAD].reshape(WPC, 128).T.astype(np.int32).copy()
        d["rp1s"] = rpv[1 : NPAD + 1].reshape(WPC, 128).T.astype(np.int32).copy()
        cores.append(d)
    return dict(K=K, CH=CH, cores=cores, rp0g=rp0g, rp1g=rp1g)


# ---------------------------------------------------------------- program
def build_program(K):
    import os
    K = np.asarray(K)
    CH = K.sum(axis=1)
    psa_bufs = int(os.environ.get("PSA_BUFS", "2"))
    nc = bacc.Bacc(None, target_bir_lowering=False, debug=False)

    x_p = nc.dram_tensor("x", [NPAD_G, D], F32, kind="ExternalInput")
    xs_p = nc.dram_tensor("xs", [NPAD, D], F32, kind="ExternalInput")
    wt_p = nc.dram_tensor("wt", [D, D], F32, kind="ExternalInput")
    iota_p = nc.dram_tensor("iota", [128, 128], F32, kind="ExternalInput")
    ident_p = nc.dram_tensor("ident", [128, 128], F32, kind="ExternalInput")
    rp0g_p = nc.dram_tensor("rp0g", [128, PT_G], I32, kind="ExternalInput")
    rp1g_p = nc.dram_tensor("rp1g", [128, PT_G], I32, kind="ExternalInput")
    rp0s_p = nc.dram_tensor("rp0s", [128, WPC], I32, kind="ExternalInput")
    rp1s_p = nc.dram_tensor("rp1s", [128, WPC], I32, kind="ExternalInput")
    gidx_p = [
        nc.dram_tensor("gidx_lo", [128, int(CH[0]) * 8], I16, kind="ExternalInput"),
        nc.dram_tensor("gidx_hi", [128, int(CH[1]) * 8], I16, kind="ExternalInput"),
    ]
    slots_p = [
        nc.dram_tensor("slots_lo", [128, int(CH[0])], I16, kind="ExternalInput"),
        nc.dram_tensor("slots_hi", [128, int(CH[1])], I16, kind="ExternalInput"),
    ]
    out_p = nc.dram_tensor("out", [NPAD, D], F32, kind="ExternalOutput")
    h_lo_t = nc.dram_tensor("h_lo", [SPLIT, D], F32)
    h_hi_t = nc.dram_tensor("h_hi", [NPAD_G - SPLIT, D], F32)

    with tile.TileContext(nc) as tc:
        with (
            tc.tile_pool(name="const", bufs=1) as cpool,
            tc.tile_pool(name="hpass", bufs=3) as hpool,
            tc.tile_pool(name="gather", bufs=8) as gpool,
            tc.tile_pool(name="meta", bufs=2) as mpool,
            tc.tile_pool(name="sel", bufs=8) as spool,
            tc.tile_pool(name="fin", bufs=3) as fpool,
            tc.tile_pool(name="psA", bufs=psa_bufs, space="PSUM") as psA,
            tc.tile_pool(name="psT", bufs=2, space="PSUM") as psT,
            tc.tile_pool(name="psO", bufs=2, space="PSUM") as psO,
        ):
            # --- constants
            wt_sb = cpool.tile([128, 128], F32, tag="wt")
            nc.sync.dma_start(wt_sb[:], wt_p[:])
            iota_sb = cpool.tile([128, 128], F32, tag="iota")
            nc.sync.dma_start(iota_sb[:], iota_p[:])
            ident_sb = cpool.tile([128, 128], F32, tag="ident")
            nc.sync.dma_start(ident_sb[:], ident_p[:])

            # --- prefetch gather indices + slot ids (ahead of h-pass in the
            # sync DMA FIFO so the first gathers aren't queued behind it)
            gidx_sb, stf = [], []
            for p in range(2):
                gi = cpool.tile([128, int(CH[p]) * 8], I16, tag=f"gidx{p}")
                nc.sync.dma_start(gi[:], gidx_p[p][:])
                si = mpool.tile([128, int(CH[p])], I16, tag="si")
                nc.sync.dma_start(si[:], slots_p[p][:])
                sf = cpool.tile([128, int(CH[p])], F32, tag=f"sf{p}")
                nc.vector.tensor_copy(sf[:], si[:])
                gidx_sb.append(gi)
                stf.append(sf)

            xs_v = xs_p[:].rearrange("(u p) d -> p u d", p=128)
            xsw = cpool.tile([128, NPAD], F32, tag="xsw")
            nc.sync.dma_start(
                out=xsw[:].rearrange("p (u e) -> p u e", e=128), in_=xs_v[:, :, :]
            )

            # --- dis = 1/sqrt(deg+1) from rowptr diffs
            def compute_dis(rp0_param, rp1_param, T, tag):
                r0i = cpool.tile([128, T], I32, tag=f"{tag}_r0i")
                nc.sync.dma_start(r0i[:], rp0_param[:])
                r1i = cpool.tile([128, T], I32, tag=f"{tag}_r1i")
                nc.sync.dma_start(r1i[:], rp1_param[:])
                r0f = cpool.tile([128, T], F32, tag=f"{tag}_r0f")
                nc.vector.tensor_copy(r0f[:], r0i[:])
                r1f = cpool.tile([128, T], F32, tag=f"{tag}_r1f")
                nc.vector.tensor_copy(r1f[:], r1i[:])
                dg = cpool.tile([128, T], F32, tag=f"{tag}_dg")
                nc.vector.tensor_tensor(out=dg[:], in0=r1f[:], in1=r0f[:], op=OP.subtract)
                nc.vector.tensor_scalar_add(out=dg[:], in0=dg[:], scalar1=1.0)
                rc = cpool.tile([128, T], F32, tag=f"{tag}_rc")
                nc.vector.reciprocal(rc[:], dg[:])
                ds = cpool.tile([128, T], F32, tag=f"{tag}_dis")
                nc.scalar.activation(ds[:], rc[:], AF.Sqrt)
                return ds

            dis_g = compute_dis(rp0g_p, rp1g_p, PT_G, "g")
            dis_s = compute_dis(rp0s_p, rp1s_p, WPC, "s")

            # --- h = x * dis; hi half first so pass-hi gathers start early
            TSPLIT = SPLIT // 128  # 256, multiple of TB
            x_v = x_p[:].rearrange("(t p) d -> p t d", p=128)
            h_lo_v = h_lo_t[:].rearrange("(t p) d -> p t d", p=128)
            h_hi_v = h_hi_t[:].rearrange("(t p) d -> p t d", p=128)
            t0_order = list(range(TSPLIT, PT_G, TB)) + list(range(0, TSPLIT, TB))
            for t0 in t0_order:
                nb = min(TB, PT_G - t0)
                xt = hpool.tile([128, TB * 128], F32, tag="xt")
                nc.sync.dma_start(
                    out=xt[:, : nb * 128].rearrange("p (b e) -> p b e", e=128),
                    in_=x_v[:, t0 : t0 + nb, :],
                )
                ht = hpool.tile([128, TB * 128], F32, tag="ht")
                for j in range(nb):
                    nc.scalar.activation(
                        ht[:, j * 128 : (j + 1) * 128],
                        xt[:, j * 128 : (j + 1) * 128],
                        AF.Copy,
                        scale=dis_g[:, t0 + j : t0 + j + 1],
                    )
                hv = h_lo_v if t0 < TSPLIT else h_hi_v
                tb = t0 if t0 < TSPLIT else t0 - TSPLIT
                nc.sync.dma_start(
                    out=hv[:, tb : tb + nb, :],
                    in_=ht[:, : nb * 128].rearrange("p (b e) -> p b e", e=128),
                )

            # --- aggregation, pass-major: hi first (its table half is
            # written first). Finalize runs as a separate phase after both
            # passes (concurrent finalize proved unstable on HW).
            SB = GB
            tables = [h_lo_t, h_hi_t]
            agg_sb = cpool.tile([128, NPAD], F32, tag="agg")

            for p in (1, 0):
                table = tables[p]
                nch = int(CH[p])
                pos = 0
                for u in range(WPC):
                    Ku = int(K[p][u])
                    sl = slice(u * 128, (u + 1) * 128)
                    if Ku == 0:
                        if p == 1:
                            nc.vector.memset(agg_sb[:, sl], 0.0)
                        else:
                            nc.vector.tensor_tensor(
                                out=agg_sb[:, sl], in0=agg_sb[:, sl], in1=xsw[:, sl], op=OP.add)
                            nc.scalar.activation(
                                agg_sb[:, sl], agg_sb[:, sl], AF.Copy,
                                scale=dis_s[:, u : u + 1])
                        continue
                    ps = psA.tile([128, 128], F32, tag="pacc")
                    for kin in range(Ku):
                        g = pos
                        pos += 1
                        b, kk = divmod(g, SB)
                        if kk == 0:
                            b0 = b * SB
                            B = min(SB, nch - b0)
                            gt = gpool.tile([128, SB * 128], F32, tag="gt")
                            gv = gt[:, : B * 128].rearrange("p (b e) -> p b e", e=128)
                            nc.gpsimd.dma_gather(
                                gv, table[:], gidx_sb[p][:, b0 * 8 : (b0 + B) * 8],
                                B * 128, B * 128, 128,
                            )
                            Sw = spool.tile([128, SB * 128], F32, tag="S")
                            base = stf[p][:, b0 : b0 + B]
                            in0 = bass.AP(base.tensor, base.offset, list(base.ap) + [[0, 128]])
                            ii = iota_sb[:]
                            in1 = bass.AP(ii.tensor, ii.offset, [ii.ap[0], [0, B], ii.ap[1]])
                            nc.vector.tensor_tensor(
                                out=Sw[:, : B * 128].rearrange("p (b e) -> p b e", e=128),
                                in0=in0, in1=in1, op=OP.is_equal,
                            )
                        nc.tensor.matmul(
                            ps[:],
                            lhsT=Sw[:, kk * 128 : (kk + 1) * 128],
                            rhs=gt[:, kk * 128 : (kk + 1) * 128],
                            start=(kin == 0),
                            stop=(kin == Ku - 1),
                        )
                    if p == 1:
                        nc.vector.tensor_copy(agg_sb[:, sl], ps[:])
                    else:
                        nc.vector.tensor_tensor(out=agg_sb[:, sl], in0=agg_sb[:, sl], in1=ps[:], op=OP.add)
                        # fold (+x, *dis) in here so these SBUF-only ops ride
                        # the in-order DVE/ACT streams instead of queuing
                        # after the whole aggregation phase
                        nc.vector.tensor_tensor(
                            out=agg_sb[:, sl], in0=agg_sb[:, sl], in1=xsw[:, sl], op=OP.add)
                        nc.scalar.activation(
                            agg_sb[:, sl], agg_sb[:, sl], AF.Copy,
                            scale=dis_s[:, u : u + 1])

            # --- finalize phase: transpose + linear + relu + store only
            out_v = out_p[:].rearrange("(u p) d -> p u d", p=128)
            for u in range(WPC):
                sl = slice(u * 128, (u + 1) * 128)
                pt = psT.tile([128, 128], F32, tag="pt")
                nc.tensor.transpose(pt[:], agg_sb[:, sl], ident_sb[:])
                att = fpool.tile([128, 128], F32, tag="fat")
                nc.scalar.copy(att[:], pt[:])
                po = psO.tile([128, 128], F32, tag="po")
                nc.tensor.matmul(po[:], lhsT=att[:], rhs=wt_sb[:], start=True, stop=True)
                ot = fpool.tile([128, 128], F32, tag="fo")
                nc.scalar.activation(ot[:], po[:], AF.Relu)
                nc.sync.dma_start(out_v[:, u, :], ot[:])

    nc.compile()
    return nc


# ---------------------------------------------------------------- runner
_CACHE = {}


def _get_program(K):
    key = K.tobytes()
    if key not in _CACHE:
        _CACHE[key] = build_program(K)
    return _CACHE[key]


def make_in_maps(x, W, prep):
    x = np.asarray(x, np.float32)
    Wt = np.ascontiguousarray(np.asarray(W, np.float32).T)
    xpad = np.zeros((NPAD_G, D), np.float32)
    xpad[:N] = x
    iota = np.tile(np.arange(128, dtype=np.float32)[None, :], (128, 1))
    ident = np.eye(128, dtype=np.float32)
    in_maps = []
    for c in range(C):
        cd = prep["cores"][c]
        xs = np.zeros((NPAD, D), np.float32)
        xs[:NPC] = x[c * NPC : (c + 1) * NPC]
        in_maps.append(
            {
                "x": xpad,
                "xs": xs,
                "wt": Wt,
                "iota": iota,
                "ident": ident,
                "rp0g": prep["rp0g"],
                "rp1g": prep["rp1g"],
                "rp0s": cd["rp0s"],
                "rp1s": cd["rp1s"],
                "gidx_lo": cd["gidx_lo"],
                "gidx_hi": cd["gidx_hi"],
                "slots_lo": cd["slots_lo"],
                "slots_hi": cd["slots_hi"],
            }
        )
    return in_maps


def run_spmd(x, edge_index, W, trace=False, **spmd_kwargs):
    prep = host_prep(edge_index)
    nc = _get_program(prep["K"])
    in_maps = make_in_maps(x, W, prep)
    res = run_bass_kernel_spmd(nc, in_maps, list(range(C)), trace=trace, **spmd_kwargs)
    out = np.concatenate([res.results[c]["out"][:NPC] for c in range(C)], axis=0)
    return out.astype(np.float32), res


def kernel(x, edge_index, N=None, W=None, **_):
    out, _res = run_spmd(np.asarray(x), np.asarray(edge_index), np.asarray(W))
    return out

